# revision 15
# baseline (speedup 1.0000x reference)
"""w8a8 int8 linear (x @ qweight) * input_scale * weight_scale on 8 trn2 cores.

Column-parallel sharding: qweight/weight_scale split along N across the 8
cores, x replicated, each core produces its own [M, N/8] output slab.

Device kernel (per core):
  - quantize x on DVE: t = x*(1/s) + 1.5*2^23  (magic round-to-nearest-even)
         t = max(t - 1.5*2^23, -128); t = min(t, 127) -> bf16 (exact integer)
  - DMA-xbar transpose [128, K] bf16 -> k-major tiles for the matmul lhsT
  - bf16 matmul accumulating fp32 in PSUM: int8-exact (|acc| << 2^24)
  - dequant on DVE: psum * (input_scale*weight_scale[n]) -> fp32 out
"""

import numpy as np
import ml_dtypes

import concourse.bass as bass
import concourse.mybir as mybir
import concourse.tile as tile
from concourse.bass_utils import run_bass_kernel_spmd

M, K, N_TOTAL, N_CORES = 8192, 4096, 11008, 8
NSH = N_TOTAL // N_CORES  # 1376 columns per core
MAGIC = 12582912.0  # 1.5 * 2**23: fp32 add rounds-to-nearest-even to integer

F32 = mybir.dt.float32
BF16 = mybir.dt.bfloat16


def split_excess_waits(nc: bass.Bass, max_waits: int = 1) -> int:
    """The walrus build here encodes at most one sync wait per instruction;
    hoist extras onto same-engine NoOps inserted before the offending inst."""
    n_split = 0
    for f in nc.m.functions:
        for blk in f.blocks:
            out = []
            changed = False
            for inst in blk.instructions:
                si = inst.sync_info
                if si is not None and si.on_wait and len(si.on_wait) > max_waits:
                    waits = list(si.on_wait)
                    extra, keep = waits[:-max_waits], waits[-max_waits:]
                    while extra:
                        chunk, extra = extra[:max_waits], extra[max_waits:]
                        out.append(
                            mybir.InstNoOp(
                                name=nc.get_next_instruction_name(),
                                engine=inst.engine,
                                sync_info=mybir.SyncInfo(on_wait=chunk, on_update=[]),
                                text_hint="wait_split",
                            )
                        )
                        n_split += 1
                    si.on_wait = keep
                    changed = True
                out.append(inst)
            if changed:
                blk.instructions = out
    return n_split


def build_nc(inv_s: float, m: int = M, k: int = K, nsh: int = NSH) -> bass.Bass:
    assert m % 128 == 0 and k % 128 == 0
    m_tiles, k_tiles = m // 128, k // 128
    # n-tile split: 512-wide chunks (one PSUM bank each) + remainder
    n_tiles = []
    off = 0
    while off < nsh:
        w = min(512, nsh - off)
        n_tiles.append((off, w))
        off += w

    nc = bass.Bass()
    x = nc.dram_tensor("x", [m, k], F32, kind="ExternalInput")
    qw = nc.dram_tensor("qw", [k, nsh], BF16, kind="ExternalInput")
    scale = nc.dram_tensor("scale", [128, nsh], F32, kind="ExternalInput")
    out = nc.dram_tensor("out", [m, nsh], BF16, kind="ExternalOutput")

    assert m_tiles % 2 == 0
    with tile.TileContext(nc) as tc:
        with (
            tc.tile_pool(name="const", bufs=1) as const_pool,
            tc.tile_pool(name="xin_p", bufs=2) as xin_pool,
            tc.tile_pool(name="xqb_p", bufs=2) as xqb_pool,
            tc.tile_pool(name="xt_p", bufs=2) as xt_pool,
            tc.tile_pool(name="out_p", bufs=2) as out_pool,
            tc.tile_pool(name="psum", bufs=2, space="PSUM") as psum_pool,
        ):
            # Prefetch the first two x tiles ahead of the qweight preload so
            # the quantize/transpose pipeline starts immediately; stream the
            # 11MB qweight in chunks alternating between the two HWDGE rings
            # so neither ring blocks the x path for long.
            xin_pre = []
            for mi in range(2):
                xin = xin_pool.tile([128, k], F32, name="xin")
                nc.sync.dma_start(xin, x.ap()[mi * 128 : (mi + 1) * 128, :])
                xin_pre.append(xin)

            # qweight resident in SBUF, k-tiled: qw_sb[p, g, n] = qw[g*128+p, n]
            # All head-phase copies go on the SP ring, which executes FIFO:
            # the x prefetch above drains at full HBM rate before qw starts,
            # and the ACT ring carries ONLY the xbar transposes (a copy on
            # that ring would serialize against them via the xbar-mode bug).
            qw_sb = const_pool.tile([128, k_tiles, nsh], BF16)
            qw_kmaj = qw.ap().rearrange("(g p) n -> p g n", p=128)
            gchunk = max(1, k_tiles // 2)
            for g0 in range(0, k_tiles, gchunk):
                g1 = min(g0 + gchunk, k_tiles)
                nc.sync.dma_start(qw_sb[:, g0:g1, :], qw_kmaj[:, g0:g1, :])
            scale_sb = const_pool.tile([128, nsh], F32)
            nc.sync.dma_start(scale_sb, scale.ap())

            # Two m-tiles share one xbar-transpose instruction: the
            # copy<->transpose xbar-mode workaround serializes the whole DMA
            # system pairwise, so fewer/bigger transposes shorten the
            # serialized DMA chain per pair of tiles.
            for pair in range(m_tiles // 2):
                xqb2 = xqb_pool.tile([128, 2, k], BF16, name="xqb2")
                for j in range(2):
                    mi = 2 * pair + j
                    if mi < 2:
                        xin = xin_pre[mi]
                    else:
                        xin = xin_pool.tile([128, k], F32, name="xin")
                        nc.sync.dma_start(
                            xin, x.ap()[mi * 128 : (mi + 1) * 128, :]
                        )
                    # quantize: round(clip(x/s,-128,127)) as exact bf16 int
                    # (passes A+B in-place in xin)
                    nc.vector.tensor_scalar(
                        xin, xin, inv_s, MAGIC,
                        mybir.AluOpType.mult, mybir.AluOpType.add,
                    )
                    nc.vector.tensor_scalar(
                        xin, xin, MAGIC, -128.0,
                        mybir.AluOpType.subtract, mybir.AluOpType.max,
                    )
                    nc.vector.tensor_scalar(
                        xqb2[:, j, :], xin, 127.0, None, mybir.AluOpType.min,
                    )
                # transpose both tiles with one xbar instruction:
                # xt2[p, j, g, m'] = xqb2[m', j, g*128+p]
                xt2 = xt_pool.tile([128, 2, k_tiles, 128], BF16, name="xt2")
                nc.scalar.dma_start_transpose(
                    xt2.rearrange("p j g m -> p (j g) m"),
                    xqb2.rearrange("p j k -> p (j k)"),
                )

                for j in range(2):
                    mi = 2 * pair + j
                    psums = [
                        psum_pool.tile([128, 512], F32, name=f"ps{t}")[:, :w]
                        for t, (o, w) in enumerate(n_tiles)
                    ]
                    for g in range(k_tiles):
                        lhsT = xt2[:, j, g, :]
                        for t, (o, w) in enumerate(n_tiles):
                            nc.tensor.matmul(
                                psums[t], lhsT, qw_sb[:, g, o : o + w],
                                start=(g == 0), stop=(g == k_tiles - 1),
                            )

                    osb = out_pool.tile([128, nsh], BF16, name="osb")
                    for t, (o, w) in enumerate(n_tiles):
                        nc.vector.tensor_tensor(
                            osb[:, o : o + w], psums[t], scale_sb[:, o : o + w],
                            mybir.AluOpType.mult,
                        )
                    nc.sync.dma_start(out.ap()[mi * 128 : (mi + 1) * 128, :], osb)

    split_excess_waits(nc)
    return nc


def kernel(x, qweight, weight_scale, input_scale, _trace=False, _tmpdir=None):
    x = np.ascontiguousarray(np.asarray(x, dtype=np.float32))
    qweight = np.asarray(qweight)
    if qweight.dtype != np.int8:
        qweight = qweight.astype(np.int8)
    weight_scale = np.asarray(weight_scale, dtype=np.float32)
    s = np.float32(np.asarray(input_scale).reshape(-1)[0])
    inv_s = float(np.float32(1.0) / s)

    nc = build_nc(inv_s)

    in_maps = []
    for c in range(N_CORES):
        sl = slice(c * NSH, (c + 1) * NSH)
        comb = (s * weight_scale[sl]).astype(np.float32)
        scale_bc = np.ascontiguousarray(np.broadcast_to(comb[None, :], (128, NSH)))
        in_maps.append({
            "x": x,
            "qw": np.ascontiguousarray(qweight[:, sl].astype(ml_dtypes.bfloat16)),
            "scale": scale_bc,
        })

    res = run_bass_kernel_spmd(
        nc, in_maps, core_ids=list(range(N_CORES)),
        trace=_trace, tmpdir=_tmpdir,
    )
    out = np.concatenate(
        [np.asarray(r["out"]).astype(np.float32) for r in res.results], axis=1
    )
    if _trace:
        return out, res
    return out



# revision 18
# speedup vs baseline: 1.0483x; 1.0483x over previous
"""w8a8 int8 linear (x @ qweight) * input_scale * weight_scale on 8 trn2 cores.

Column-parallel sharding: qweight/weight_scale split along N across the 8
cores, x replicated, each core produces its own [M, N/8] output slab.

Device kernel (per core):
  - quantize x on DVE: t = x*(1/s) + 1.5*2^23  (magic round-to-nearest-even)
         t = max(t - 1.5*2^23, -128); t = min(t, 127) -> bf16 (exact integer)
  - DMA-xbar transpose [128, K] bf16 -> k-major tiles for the matmul lhsT
  - bf16 matmul accumulating fp32 in PSUM: int8-exact (|acc| << 2^24)
  - dequant on DVE: psum * (input_scale*weight_scale[n]) -> bf16 out slab
    (upcast to fp32 on the host; norm rel err ~1.7e-3 vs the 2e-2 gate)

Perf notes (measured on HW, 8-core SPMD, max core time):
  - PE roofline for the per-core GEMM slab at bf16 (the only dtype that is
    int8-exact at full PE rate) is 64*32*1376 cycles @2.4GHz = 1.174 ms.
  - v1 baseline 1.79 ms: per-m-tile producer chain (x load 5.9us + 3 DVE
    passes 6.9us + 8 chunked xbar transposes 10.4us, all on one HWDGE ring)
    exceeded the PE's 18.3us/tile -> 7-9us PE stall + HAM re-throttle per tile.
  - 1.28 ms now: one transpose instruction per tile on the ACT ring, copies
    on the SP ring, bufs=3 pipeline. Remaining over roofline: ~50us head
    (qweight preload serializes with the x pipeline via the xbar-mode
    copy<->transpose workaround + 8 rotating DMA-completion sem lanes),
    ~40us of ~365ns SDMA/SBUF contention hiccups, ~13us drain tail.
    Head reorderings (rings, chunking, SWDGE int8+cast, 2-tile transposes)
    all measured neutral-or-worse; the serialized DMA chain self-regulates
    to ~the same total.
"""

import numpy as np
import ml_dtypes

import concourse.bass as bass
import concourse.mybir as mybir
import concourse.tile as tile
from concourse.bass_utils import run_bass_kernel_spmd

M, K, N_TOTAL, N_CORES = 8192, 4096, 11008, 8
NSH = N_TOTAL // N_CORES  # 1376 columns per core
MAGIC = 12582912.0  # 1.5 * 2**23: fp32 add rounds-to-nearest-even to integer

F32 = mybir.dt.float32
BF16 = mybir.dt.bfloat16


def split_excess_waits(nc: bass.Bass, max_waits: int = 1) -> int:
    """The walrus build here encodes at most one sync wait per instruction;
    hoist extras onto same-engine NoOps inserted before the offending inst."""
    n_split = 0
    for f in nc.m.functions:
        for blk in f.blocks:
            out = []
            changed = False
            for inst in blk.instructions:
                si = inst.sync_info
                if si is not None and si.on_wait and len(si.on_wait) > max_waits:
                    waits = list(si.on_wait)
                    extra, keep = waits[:-max_waits], waits[-max_waits:]
                    while extra:
                        chunk, extra = extra[:max_waits], extra[max_waits:]
                        out.append(
                            mybir.InstNoOp(
                                name=nc.get_next_instruction_name(),
                                engine=inst.engine,
                                sync_info=mybir.SyncInfo(on_wait=chunk, on_update=[]),
                                text_hint="wait_split",
                            )
                        )
                        n_split += 1
                    si.on_wait = keep
                    changed = True
                out.append(inst)
            if changed:
                blk.instructions = out
    return n_split


def build_nc(inv_s: float, m: int = M, k: int = K, nsh: int = NSH) -> bass.Bass:
    assert m % 128 == 0 and k % 128 == 0
    m_tiles, k_tiles = m // 128, k // 128
    # n-tile split: 512-wide chunks (one PSUM bank each) + remainder
    n_tiles = []
    off = 0
    while off < nsh:
        w = min(512, nsh - off)
        n_tiles.append((off, w))
        off += w

    nc = bass.Bass()
    x = nc.dram_tensor("x", [m, k], F32, kind="ExternalInput")
    qw = nc.dram_tensor("qw", [k, nsh], BF16, kind="ExternalInput")
    scale = nc.dram_tensor("scale", [128, nsh], F32, kind="ExternalInput")
    out = nc.dram_tensor("out", [m, nsh], BF16, kind="ExternalOutput")

    with tile.TileContext(nc) as tc:
        with (
            tc.tile_pool(name="const", bufs=1) as const_pool,
            tc.tile_pool(name="xin_p", bufs=3) as xin_pool,
            tc.tile_pool(name="xqb_p", bufs=3) as xqb_pool,
            tc.tile_pool(name="xt_p", bufs=3) as xt_pool,
            tc.tile_pool(name="out_p", bufs=2) as out_pool,
            tc.tile_pool(name="psum", bufs=2, space="PSUM") as psum_pool,
        ):
            # Prefetch the first two x tiles ahead of the qweight preload so
            # the quantize/transpose pipeline starts immediately; stream the
            # 11MB qweight in chunks alternating between the two HWDGE rings
            # so neither ring blocks the x path for long.
            xin_pre = []
            for mi in range(2):
                xin = xin_pool.tile([128, k], F32, name="xin")
                nc.sync.dma_start(xin, x.ap()[mi * 128 : (mi + 1) * 128, :])
                xin_pre.append(xin)

            # qweight resident in SBUF, k-tiled: qw_sb[p, g, n] = qw[g*128+p, n]
            # All head-phase copies go on the SP ring, which executes FIFO:
            # the x prefetch above drains at full HBM rate before qw starts,
            # and the ACT ring carries ONLY the xbar transposes (a copy on
            # that ring would serialize against them via the xbar-mode bug).
            qw_sb = const_pool.tile([128, k_tiles, nsh], BF16)
            qw_kmaj = qw.ap().rearrange("(g p) n -> p g n", p=128)
            gchunk = max(1, k_tiles // 2)
            for g0 in range(0, k_tiles, gchunk):
                g1 = min(g0 + gchunk, k_tiles)
                nc.sync.dma_start(qw_sb[:, g0:g1, :], qw_kmaj[:, g0:g1, :])
            scale_sb = const_pool.tile([128, nsh], F32)
            nc.sync.dma_start(scale_sb, scale.ap())

            for mi in range(m_tiles):
                if mi < 2:
                    xin = xin_pre[mi]
                else:
                    xin = xin_pool.tile([128, k], F32, name="xin")
                    nc.sync.dma_start(xin, x.ap()[mi * 128 : (mi + 1) * 128, :])
                # quantize: round(clip(x/s, -128, 127)) as exact bf16 integer
                # (passes A+B in-place in xin to save SBUF for deeper bufs)
                nc.vector.tensor_scalar(
                    xin, xin, inv_s, MAGIC,
                    mybir.AluOpType.mult, mybir.AluOpType.add,
                )
                nc.vector.tensor_scalar(
                    xin, xin, MAGIC, -128.0,
                    mybir.AluOpType.subtract, mybir.AluOpType.max,
                )
                xqb = xqb_pool.tile([128, k], BF16, name="xqb")
                nc.vector.tensor_scalar(
                    xqb, xin, 127.0, None, mybir.AluOpType.min,
                )
                # transpose via DMA xbar: xt[p, g, m'] = xqb[m', g*128+p]
                # one instruction per m-tile, on the ACT HWDGE ring so the
                # SP ring's copies queue independently of the transposes
                xt = xt_pool.tile([128, k_tiles, 128], BF16, name="xt")
                nc.scalar.dma_start_transpose(xt, xqb)

                psums = [
                    psum_pool.tile([128, 512], F32, name=f"ps{j}")[:, :w]
                    for j, (o, w) in enumerate(n_tiles)
                ]
                for g in range(k_tiles):
                    lhsT = xt[:, g, :]
                    for j, (o, w) in enumerate(n_tiles):
                        nc.tensor.matmul(
                            psums[j], lhsT, qw_sb[:, g, o : o + w],
                            start=(g == 0), stop=(g == k_tiles - 1),
                        )

                osb = out_pool.tile([128, nsh], BF16, name="osb")
                for j, (o, w) in enumerate(n_tiles):
                    nc.vector.tensor_tensor(
                        osb[:, o : o + w], psums[j], scale_sb[:, o : o + w],
                        mybir.AluOpType.mult,
                    )
                nc.sync.dma_start(out.ap()[mi * 128 : (mi + 1) * 128, :], osb)

    split_excess_waits(nc)
    return nc


def kernel(x, qweight, weight_scale, input_scale, _trace=False, _tmpdir=None):
    x = np.ascontiguousarray(np.asarray(x, dtype=np.float32))
    qweight = np.asarray(qweight)
    if qweight.dtype != np.int8:
        qweight = qweight.astype(np.int8)
    weight_scale = np.asarray(weight_scale, dtype=np.float32)
    s = np.float32(np.asarray(input_scale).reshape(-1)[0])
    inv_s = float(np.float32(1.0) / s)

    nc = build_nc(inv_s)

    in_maps = []
    for c in range(N_CORES):
        sl = slice(c * NSH, (c + 1) * NSH)
        comb = (s * weight_scale[sl]).astype(np.float32)
        scale_bc = np.ascontiguousarray(np.broadcast_to(comb[None, :], (128, NSH)))
        in_maps.append({
            "x": x,
            "qw": np.ascontiguousarray(qweight[:, sl].astype(ml_dtypes.bfloat16)),
            "scale": scale_bc,
        })

    res = run_bass_kernel_spmd(
        nc, in_maps, core_ids=list(range(N_CORES)),
        trace=_trace, tmpdir=_tmpdir,
    )
    out = np.concatenate(
        [np.asarray(r["out"]).astype(np.float32) for r in res.results], axis=1
    )
    if _trace:
        return out, res
    return out

